# revision 12
# baseline (speedup 1.0000x reference)
"""DPM matching head (cosine-shift clustering) Trainium2 kernel.

Full inputs:  prototypes [32,10,768] f32, feats [32,4096,768] f32,
              feats_org [4096,768] f32.
Returns the reference tuple:
  prototypes_out [320,768] f32, sim_map [320,4096] f32.

Sharding: pure data parallel over the leading B=32 dim, 4 objects per
NeuronCore, 8 cores.

Precision: the 5-iteration hard-assignment loop chaotically amplifies any
matmul error (tf32-level sim error -> 3e-2 final rel err), so every matmul
runs in true fp32 (4 cycles/row on the PE).

Per-core per-object plan:
  build:  stream feats tiles [128,768] from HBM, compute 1/max(||f||,eps),
          scale in place, PE-transpose into resident D-major normalized
          FT [128, 6*4096] (96KB/partition).  Raw feats are re-streamed
          from HBM each iteration for the update matmul (fits easily under
          the fp32 PE time).
  iter t: norm chain U->Pn,PnT; density chain (prev iter's S,msum + Pn) ->
          tau; sim matmul PnT x FT -> PSUM [10,512], grouped online
          softmax (ACT Exp, fused row-sums, per-group max correction);
          DVE 32x32 stream-transpose of the weight matrix; column-max /
          equality mask in the transposed domain; spread SBUF->SBUF DMAs
          into the N-major lhsT layout; one fused update matmul (M=30:
          new prototypes | mask*r row-sums S | mask counts via a ones
          column), density dot from S next iteration.
  final:  normalized prototypes of all objects gathered to [40,768],
          feats_org build (same machinery), one M=40 matmul -> sim_map.
"""

import os
import sys

for _p in ("/opt/trn_rl_repo", "/root/.axon_site/_ro/trn_rl_repo"):
    if os.path.isdir(_p) and _p not in sys.path:
        sys.path.append(_p)

import numpy as np

import concourse.bass as bass
import concourse.tile as tile
from concourse import mybir

FP = mybir.dt.float32
AF = mybir.ActivationFunctionType
ALU = mybir.AluOpType
AXX = mybir.AxisListType.X

TEMP = 0.1
TAU0 = 0.1
EPS = 1e-8
TAU_MIN = 1e-10


def build_kernel(nobj=4, N=4096, D=768, P=10, n_iters=5, debug=False):
    """Emit the single-core bass program (same program runs SPMD on all cores)."""
    assert N % 512 == 0 and D % 128 == 0 and P <= 10
    NT = N // 128          # feats 128-row tiles == update K-chunks
    DT = D // 128          # D-chunks for the sim contraction
    NB = N // 512          # 512-wide sim free chunks
    GRP = 4                # sim psum banks in flight (online-softmax group)
    NG = (NB + GRP - 1) // GRP
    M3 = 3 * P             # update lhsT columns: W | mask*r | mask
    MA = nobj * P          # final matmul M
    DSPLITS = [(0, 512), (512, D)] if D > 512 else [(0, D)]

    nc = bass.Bass()

    protos_in = nc.declare_dram_parameter("prototypes", [nobj, P, D], FP, isOutput=False)
    feats_in = nc.declare_dram_parameter("feats", [nobj, N, D], FP, isOutput=False)
    forg_in = nc.declare_dram_parameter("feats_org", [N, D], FP, isOutput=False)
    ident_in = nc.declare_dram_parameter("identity", [128, 128], FP, isOutput=False)
    protos_out = nc.declare_dram_parameter("protos_out", [nobj, P, D], FP, isOutput=True)
    simmap_out = nc.declare_dram_parameter("sim_map_out", [nobj, P, N], FP, isOutput=True)
    dbg_out = (nc.declare_dram_parameter("dbg_out", [nobj, n_iters, 8, 32], FP,
                                         isOutput=True) if debug else None)

    with tile.TileContext(nc) as tc, \
            tc.tile_pool(name="singles", bufs=1) as singles, \
            tc.tile_pool(name="win", bufs=4) as win, \
            tc.tile_pool(name="small", bufs=2) as small, \
            tc.tile_pool(name="ps", bufs=4, space="PSUM") as ps:

        ident = singles.tile([128, 128], FP, tag="ident")
        nc.sync.dma_start(out=ident, in_=ident_in[:, :])
        ones = singles.tile([128, 1], FP, tag="ones")
        nc.vector.memset(ones, 1.0)

        FT = singles.tile([128, DT * N], FP, tag="FT")          # D-major normalized feats
        FTv = FT.rearrange("p (k n) -> p k n", k=DT)
        q = singles.tile([32, N], FP, tag="q")                  # softmax weights (P rows)
        nc.vector.memset(q, -1.0)                               # pad rows: never win
        QT = singles.tile([32, N], FP, tag="QT")                # 32x32-transposed q
        QTv = QT.rearrange("p (c j) -> p c j", j=32)
        QTs = QT.rearrange("p (k r) -> p k r", r=128)           # free = 128k + 32d + j
        colmax = singles.tile([32, N // 32], FP, tag="colmax")
        maskT = singles.tile([32, (N // 32) * P], FP, tag="maskT")
        maskTv = maskT.rearrange("p (c j) -> p c j", j=P)
        maskTs = maskT.rearrange("p (k d j) -> p k d j", d=4, j=P)
        WT = singles.tile([128, NT * M3], FP, tag="WT")         # update lhsT
        WTv = WT.rearrange("p (k m) -> p k m", m=M3)
        rr = singles.tile([128, NT], FP, tag="rr")              # 1/max(||f||,eps)
        U = singles.tile([32, D + 4], FP, tag="U")              # current/updated protos
        Pn = singles.tile([32, D], FP, tag="Pn")                # normalized protos
        PnT = singles.tile([128, DT * P], FP, tag="PnT")        # sim lhsT
        PaT = singles.tile([128, DT * MA], FP, tag="PaT")       # final sim_map lhsT
        Pall = singles.tile([MA, D], FP, tag="Pall")
        S_al = singles.tile([32, D], FP, tag="S_al")
        msum_al = singles.tile([32, 1], FP, tag="msum_al")
        scratch = singles.tile([32, D], FP, tag="scratch")
        tau = singles.tile([32, 1], FP, tag="tau")
        zm8 = singles.tile([32, NB], FP, tag="zm8")             # per-bank sim maxes
        Z8 = singles.tile([32, NB], FP, tag="Z8")               # per-bank exp sums
        hmax = singles.tile([32, NG], FP, tag="hmax")
        hbias = singles.tile([32, NG], FP, tag="hbias")
        corr = singles.tile([32, NG], FP, tag="corr")
        scl = singles.tile([32, NG], FP, tag="scl")
        Zg = singles.tile([32, NG], FP, tag="Zg")
        st = {k: singles.tile([32, 1], FP, tag=f"st_{k}", name=f"st_{k}")
              for k in ("smax", "s", "gbias", "Z", "invZ", "ss", "nrm", "rp",
                        "dd", "m1", "mrec", "gate", "d1")}

        def build_FT(src_ap, store_r):
            """Stream [N,D] rows from DRAM, l2-normalize, transpose into FT."""
            for t in range(NT):
                w = win.tile([128, D], FP, tag="wtile")
                nc.sync.dma_start(out=w, in_=src_ap[t * 128:(t + 1) * 128, :])
                bs = small.tile([128, D], FP, tag="bscr")
                bq = small.tile([128, 1], FP, tag="bssq")
                br = small.tile([128, 1], FP, tag="br")
                nc.scalar.activation(out=bs, in_=w, func=AF.Square, accum_out=bq)
                nc.scalar.sqrt(out=bq, in_=bq)
                nc.vector.tensor_scalar_max(out=bq, in0=bq, scalar1=EPS)
                nc.vector.reciprocal(out=br, in_=bq)
                nc.scalar.mul(out=w, in_=w, mul=br)
                if store_r:
                    nc.vector.tensor_copy(out=rr[:, t:t + 1], in_=br)
                for j in range(DT):
                    pt = ps.tile([128, 128], FP, tag="ps1")
                    nc.tensor.transpose(pt, w[:, j * 128:(j + 1) * 128], ident)
                    nc.vector.tensor_copy(out=FTv[:, j, t * 128:(t + 1) * 128], in_=pt)

        def norm_chain(n_rows, src, dstPn, dstPnT, id_sl):
            """src [n_rows, D] -> normalized dstPn and transposed dstPnT."""
            nc.scalar.activation(out=scratch[0:n_rows, :], in_=src, func=AF.Square,
                                 accum_out=st["ss"][0:n_rows])
            nc.scalar.sqrt(out=st["nrm"][0:n_rows], in_=st["ss"][0:n_rows])
            nc.vector.tensor_scalar_max(out=st["nrm"][0:n_rows],
                                        in0=st["nrm"][0:n_rows], scalar1=EPS)
            nc.vector.reciprocal(out=st["rp"][0:n_rows], in_=st["nrm"][0:n_rows])
            nc.scalar.mul(out=dstPn[0:n_rows, :], in_=src, mul=st["rp"][0:n_rows])
            for j in range(DT):
                pt = ps.tile([128, n_rows], FP, tag="ps1")
                nc.tensor.transpose(pt, dstPn[0:n_rows, j * 128:(j + 1) * 128], id_sl)
                nc.vector.tensor_copy(out=dstPnT[:, j * n_rows:(j + 1) * n_rows], in_=pt)

        id10 = ident[0:P, 0:P]
        idMA = ident[0:MA, 0:MA]

        for b in range(nobj):
            build_FT(feats_in[b], store_r=True)
            nc.sync.dma_start(out=U[0:P, 0:D], in_=protos_in[b])
            nc.vector.memset(tau[0:P], TAU0)

            for t in range(n_iters):
                norm_chain(P, U[0:P, 0:D], Pn, PnT, id10)

                if t > 0:
                    # density/tau from previous iteration's S, msum and new Pn
                    nc.vector.tensor_mul(out=scratch[0:P, :], in0=Pn[0:P, :],
                                         in1=S_al[0:P, :])
                    nc.vector.tensor_reduce(out=st["dd"][0:P], in_=scratch[0:P, :],
                                            axis=AXX, op=ALU.add)
                    nc.vector.tensor_scalar_max(out=st["m1"][0:P], in0=msum_al[0:P],
                                                scalar1=1.0)
                    nc.vector.reciprocal(out=st["mrec"][0:P], in_=st["m1"][0:P])
                    nc.vector.tensor_scalar_min(out=st["gate"][0:P], in0=msum_al[0:P],
                                                scalar1=1.0)
                    nc.vector.tensor_mul(out=st["d1"][0:P], in0=st["dd"][0:P],
                                         in1=st["mrec"][0:P])
                    nc.vector.tensor_mul(out=st["d1"][0:P], in0=st["d1"][0:P],
                                         in1=st["gate"][0:P])
                    nc.vector.tensor_scalar(out=tau[0:P], in0=st["d1"][0:P],
                                            scalar1=-1.0, scalar2=1.0,
                                            op0=ALU.mult, op1=ALU.add)
                    nc.vector.tensor_scalar_max(out=tau[0:P], in0=tau[0:P],
                                                scalar1=TAU_MIN)

                # s = 1/(TEMP*tau), capped: beyond ~1e4 the softmax is already
                # an exact one-hot on this data, while uncapped s amplifies the
                # HW FMA cancellation residual of (sim*s - smax*s) to +-1e4,
                # overflowing exp (CoreSim's separate-rounding matmul hides it).
                nc.vector.tensor_scalar_mul(out=st["s"][0:P], in0=tau[0:P],
                                            scalar1=TEMP)
                nc.vector.reciprocal(out=st["s"][0:P], in_=st["s"][0:P])
                nc.vector.tensor_scalar_min(out=st["s"][0:P], in0=st["s"][0:P],
                                            scalar1=1.0e4)

                # ---- sim matmul + grouped online softmax ----
                for g in range(NG):
                    banks = range(g * GRP, min((g + 1) * GRP, NB))
                    pss = []
                    for nb in banks:
                        p_ = ps.tile([P, 512], FP, tag="ps1")
                        for kd in range(DT):
                            nc.tensor.matmul(p_, PnT[:, kd * P:(kd + 1) * P],
                                             FTv[:, kd, nb * 512:(nb + 1) * 512],
                                             start=(kd == 0), stop=(kd == DT - 1))
                        nc.vector.tensor_reduce(out=zm8[0:P, nb:nb + 1], in_=p_,
                                                axis=AXX, op=ALU.max)
                        pss.append(p_)
                    nc.vector.tensor_reduce(
                        out=hmax[0:P, g:g + 1],
                        in_=zm8[0:P, g * GRP:min((g + 1) * GRP, NB)],
                        axis=AXX, op=ALU.max)
                    nc.vector.tensor_scalar(out=hbias[0:P, g:g + 1],
                                            in0=hmax[0:P, g:g + 1],
                                            scalar1=st["s"][0:P], scalar2=-1.0,
                                            op0=ALU.mult, op1=ALU.mult)
                    for p_, nb in zip(pss, banks):
                        nc.scalar.activation(out=q[0:P, nb * 512:(nb + 1) * 512],
                                             in_=p_, func=AF.Exp,
                                             bias=hbias[0:P, g:g + 1],
                                             scale=st["s"][0:P],
                                             accum_out=Z8[0:P, nb:nb + 1])
                    nc.vector.tensor_reduce(
                        out=Zg[0:P, g:g + 1],
                        in_=Z8[0:P, g * GRP:min((g + 1) * GRP, NB)],
                        axis=AXX, op=ALU.add)
                # global max + per-group correction, q *= corr_g/Z
                nc.vector.tensor_reduce(out=st["smax"][0:P], in_=hmax[0:P, 0:NG],
                                        axis=AXX, op=ALU.max)
                nc.vector.tensor_scalar(out=st["gbias"][0:P], in0=st["smax"][0:P],
                                        scalar1=st["s"][0:P], scalar2=-1.0,
                                        op0=ALU.mult, op1=ALU.mult)
                nc.scalar.activation(out=corr[0:P, 0:NG], in_=hmax[0:P, 0:NG],
                                     func=AF.Exp, bias=st["gbias"][0:P],
                                     scale=st["s"][0:P])
                nc.vector.tensor_mul(out=Zg[0:P, 0:NG], in0=Zg[0:P, 0:NG],
                                     in1=corr[0:P, 0:NG])
                nc.vector.tensor_reduce(out=st["Z"][0:P], in_=Zg[0:P, 0:NG],
                                        axis=AXX, op=ALU.add)
                nc.vector.reciprocal(out=st["invZ"][0:P], in_=st["Z"][0:P])
                nc.vector.tensor_scalar_mul(out=scl[0:P, 0:NG], in0=corr[0:P, 0:NG],
                                            scalar1=st["invZ"][0:P])
                for g in range(NG):
                    lo = g * GRP * 512
                    hi = min((g + 1) * GRP, NB) * 512
                    nc.scalar.mul(out=q[0:P, lo:hi], in_=q[0:P, lo:hi],
                                  mul=scl[0:P, g:g + 1])

                # ---- transpose weights, mask in the transposed domain ----
                nc.vector.transpose(out=QT, in_=q)
                nc.vector.tensor_reduce(out=colmax, in_=QTv[:, :, 0:P],
                                        axis=AXX, op=ALU.max)
                cm_b = bass.AP(tensor=colmax.tensor, offset=colmax.offset,
                               ap=[colmax.ap[0], colmax.ap[1], [0, P]])
                nc.vector.tensor_tensor(out=maskTv, in0=QTv[:, :, 0:P], in1=cm_b,
                                        op=ALU.is_equal)
                nc.vector.tensor_mul(out=QTv[:, :, 0:P], in0=QTv[:, :, 0:P],
                                     in1=maskTv)

                last = (t == n_iters - 1)
                MU = P if last else M3
                for d in range(4):
                    nc.sync.dma_start(out=WTv[d * 32:(d + 1) * 32, :, 0:P],
                                      in_=QTs[:, :, d * 32:d * 32 + P])
                    if not last:
                        nc.sync.dma_start(
                            out=WTv[d * 32:(d + 1) * 32, :, 2 * P:3 * P],
                            in_=maskTs[:, :, d, :])
                if not last:
                    rr_b = bass.AP(tensor=rr.tensor, offset=rr.offset,
                                   ap=[rr.ap[0], rr.ap[1], [0, P]])
                    nc.vector.tensor_tensor(out=WTv[:, :, P:2 * P],
                                            in0=WTv[:, :, 2 * P:3 * P], in1=rr_b,
                                            op=ALU.mult)

                # ---- update matmul (streams raw feats from HBM) ----
                pus = [ps.tile([M3, d1 - d0], FP, tag="pupd", bufs=3, name=f"pu{i}")
                       for i, (d0, d1) in enumerate(DSPLITS)]
                puc = (ps.tile([M3, 1], FP, tag="pupd", bufs=3, name="puc")
                       if not last else None)
                for k in range(NT):
                    w = win.tile([128, D], FP, tag="wtile")
                    nc.sync.dma_start(out=w,
                                      in_=feats_in[b, k * 128:(k + 1) * 128, :])
                    lh = WTv[:, k, 0:MU]
                    for pu, (d0, d1) in zip(pus, DSPLITS):
                        nc.tensor.matmul(pu[0:MU], lh, w[:, d0:d1],
                                         start=(k == 0), stop=(k == NT - 1))
                    if puc is not None:
                        nc.tensor.matmul(puc[0:MU], lh, ones,
                                         start=(k == 0), stop=(k == NT - 1))
                for pu, (d0, d1) in zip(pus, DSPLITS):
                    nc.vector.tensor_copy(out=U[0:MU, d0:d1], in_=pu[0:MU])
                if puc is not None:
                    nc.vector.tensor_copy(out=U[0:MU, D:D + 1], in_=puc[0:MU])
                    # partition-shift S and msum down to rows 0..P
                    nc.sync.dma_start(out=S_al[0:P, 0:D], in_=U[P:2 * P, 0:D])
                    nc.sync.dma_start(out=msum_al[0:P], in_=U[2 * P:3 * P, D:D + 1])

                if dbg_out is not None:
                    for di, dsrc in enumerate((tau, st["s"], st["smax"], st["Z"],
                                               msum_al, st["dd"], st["d1"],
                                               st["rp"])):
                        nc.sync.dma_start(out=dbg_out[b, t, di, :], in_=dsrc[:, 0])

            # object epilogue: final protos out + gather normalized rows
            nc.sync.dma_start(out=protos_out[b], in_=U[0:P, 0:D])
            norm_chain(P, U[0:P, 0:D], Pn, PnT, id10)
            nc.sync.dma_start(out=Pall[b * P:(b + 1) * P, :], in_=Pn[0:P, :])

        # ---- final sim_map against feats_org ----
        build_FT(forg_in[:, :], store_r=False)
        for j in range(DT):
            pt = ps.tile([128, MA], FP, tag="ps1")
            nc.tensor.transpose(pt, Pall[:, j * 128:(j + 1) * 128], idMA)
            nc.vector.tensor_copy(out=PaT[:, j * MA:(j + 1) * MA], in_=pt)
        smv = simmap_out.rearrange("b p n -> (b p) n")
        for nb in range(NB):
            p_ = ps.tile([MA, 512], FP, tag="ps1")
            for kd in range(DT):
                nc.tensor.matmul(p_, PaT[:, kd * MA:(kd + 1) * MA],
                                 FTv[:, kd, nb * 512:(nb + 1) * 512],
                                 start=(kd == 0), stop=(kd == DT - 1))
            sg = small.tile([MA, 512], FP, tag="smstage")
            nc.vector.tensor_copy(out=sg, in_=p_)
            nc.sync.dma_start(out=smv[:, nb * 512:(nb + 1) * 512], in_=sg)

    return nc


def legalize_waits(raw: bytes) -> bytes:
    """Split multi-wait instructions into single-wait NoOp prefixes.

    The walrus build in this environment rejects any instruction whose
    sync_info carries more than one on_wait entry ("Too many sync wait
    commands"), but Tile freely emits several.  Hoisting all but the last
    wait onto preceding same-engine NoOps is semantically identical under
    in-order per-engine execution.
    """
    import json

    data = json.loads(raw)
    n = 0
    for fn in data.get("functions", []):
        for blk in fn.get("blocks", []):
            out = []
            for inst in blk.get("instructions", []):
                si = inst.get("sync_info")
                waits = (si or {}).get("on_wait") or []
                if len(waits) > 1 and inst.get("engine") not in (None, "Unassigned"):
                    for w in waits[:-1]:
                        out.append({
                            "debug": inst.get("debug", 0),
                            "engine": inst["engine"],
                            "ins": [], "outs": [],
                            "name": f"WS-{n}",
                            "opcode": "NoOp",
                            "text_hint": "waitsplit",
                            "sync_info": {"on_wait": [w], "on_update": []},
                        })
                        n += 1
                    si["on_wait"] = [waits[-1]]
                out.append(inst)
            blk["instructions"] = out
    return json.dumps(data).encode()


_CACHE = {}


def _get_nc(key):
    if key not in _CACHE:
        nc = build_kernel(*key)
        legalized = legalize_waits(nc.to_json_bytes())
        nc.to_json_bytes = lambda: legalized
        _CACHE[key] = nc
    return _CACHE[key]


def kernel(prototypes, feats, feats_org):
    from concourse.bass_utils import run_bass_kernel_spmd

    B, P, D = prototypes.shape
    N = feats.shape[1]
    n_cores = 8
    nobj = B // n_cores
    nc = _get_nc((nobj, N, D, P, 5))
    eye = np.eye(128, dtype=np.float32)
    in_maps = []
    for c in range(n_cores):
        in_maps.append({
            "prototypes": np.ascontiguousarray(
                prototypes[c * nobj:(c + 1) * nobj], dtype=np.float32),
            "feats": np.ascontiguousarray(
                feats[c * nobj:(c + 1) * nobj], dtype=np.float32),
            "feats_org": np.ascontiguousarray(feats_org, dtype=np.float32),
            "identity": eye,
        })
    res = run_bass_kernel_spmd(nc, in_maps, list(range(n_cores))).results
    protos = np.concatenate([res[c]["protos_out"] for c in range(n_cores)], 0)
    sim_map = np.concatenate([res[c]["sim_map_out"] for c in range(n_cores)], 0)
    return protos.reshape(B * P, D), sim_map.reshape(B * P, N)


# revision 22
# speedup vs baseline: 1.0826x; 1.0826x over previous
"""DPM matching head (cosine-shift clustering) Trainium2 kernel.

Full inputs:  prototypes [32,10,768] f32, feats [32,4096,768] f32,
              feats_org [4096,768] f32.
Returns the reference tuple:
  prototypes_out [320,768] f32, sim_map [320,4096] f32.

Sharding: pure data parallel over the leading B=32 dim, 4 objects per
NeuronCore, 8 cores.

Precision: the 5-iteration hard-assignment loop chaotically amplifies any
matmul error (tf32-level sim error -> 3e-2 final rel err), so every matmul
runs in true fp32 (4 cycles/row on the PE).

Per-core per-object plan:
  build:  stream feats tiles [128,768] from HBM, compute 1/max(||f||,eps),
          scale in place, PE-transpose into resident D-major normalized
          FT [128, 6*4096] (96KB/partition).  Raw feats are re-streamed
          from HBM each iteration for the update matmul (fits easily under
          the fp32 PE time).
  iter t: norm chain U->Pn,PnT; density chain (prev iter's S,msum + Pn) ->
          tau; sim matmul PnT x FT -> PSUM [10,512], grouped online
          softmax (ACT Exp, fused row-sums, per-group max correction);
          DVE 32x32 stream-transpose of the weight matrix; column-max /
          equality mask in the transposed domain; spread SBUF->SBUF DMAs
          into the N-major lhsT layout; one fused update matmul (M=30:
          new prototypes | mask*r row-sums S | mask counts via a ones
          column), density dot from S next iteration.
  final:  normalized prototypes of all objects gathered to [40,768],
          feats_org build (same machinery), one M=40 matmul -> sim_map.
"""

import os
import sys

for _p in ("/opt/trn_rl_repo", "/root/.axon_site/_ro/trn_rl_repo"):
    if os.path.isdir(_p) and _p not in sys.path:
        sys.path.append(_p)

import numpy as np

import concourse.bass as bass
import concourse.tile as tile
from concourse import mybir

FP = mybir.dt.float32
AF = mybir.ActivationFunctionType
ALU = mybir.AluOpType
AXX = mybir.AxisListType.X

TEMP = 0.1
TAU0 = 0.1
EPS = 1e-8
TAU_MIN = 1e-10


def build_kernel(nobj=4, N=4096, D=768, P=10, n_iters=5, debug=False):
    """Emit the single-core bass program (same program runs SPMD on all cores)."""
    assert N % 512 == 0 and D % 128 == 0 and P <= 10
    NT = N // 128          # feats 128-row tiles == update K-chunks
    DT = D // 128          # D-chunks for the sim contraction
    NB = N // 512          # 512-wide sim free chunks
    GRP = 4                # sim psum banks in flight (online-softmax group)
    NG = (NB + GRP - 1) // GRP
    assert NB % GRP == 0 or NG == 1, "groups must be equal-sized"
    M3 = 3 * P             # update lhsT columns: W | mask*r | mask
    MA = nobj * P          # final matmul M
    DSPLITS = [(0, 512), (512, D)] if D > 512 else [(0, D)]

    nc = bass.Bass()

    protos_in = nc.declare_dram_parameter("prototypes", [nobj, P, D], FP, isOutput=False)
    feats_in = nc.declare_dram_parameter("feats", [nobj, N, D], FP, isOutput=False)
    forg_in = nc.declare_dram_parameter("feats_org", [N, D], FP, isOutput=False)
    ident_in = nc.declare_dram_parameter("identity", [128, 128], FP, isOutput=False)
    protos_out = nc.declare_dram_parameter("protos_out", [nobj, P, D], FP, isOutput=True)
    simmap_out = nc.declare_dram_parameter("sim_map_out", [nobj, P, N], FP, isOutput=True)
    dbg_out = (nc.declare_dram_parameter("dbg_out", [nobj, n_iters, 8, 32], FP,
                                         isOutput=True) if debug else None)

    with tile.TileContext(nc) as tc, \
            tc.tile_pool(name="singles", bufs=1) as singles, \
            tc.tile_pool(name="win", bufs=8) as win, \
            tc.tile_pool(name="small", bufs=2) as small, \
            tc.tile_pool(name="ps", bufs=4, space="PSUM") as ps:

        ident = singles.tile([128, 128], FP, tag="ident")
        nc.sync.dma_start(out=ident, in_=ident_in[:, :])
        ones = singles.tile([128, 1], FP, tag="ones")
        nc.vector.memset(ones, 1.0)

        FT = singles.tile([128, DT * N], FP, tag="FT")          # D-major normalized feats
        FTv = FT.rearrange("p (k n) -> p k n", k=DT)
        NGC = N // ((NB + GRP - 1) // GRP) // 32  # 32-cols per softmax group
        # Per softmax-group copies of the pipeline tensors.  Separate tensors
        # (not slices of one) so Tile's coarse per-tile dependency tracking
        # lets group h's update matmuls start while group h+1 is still being
        # masked/spread.
        NGK = NT // NG               # 128-chunks per group
        q_g, QT_g, colmax_g, maskT_g, WT_g = [], [], [], [], []
        for h in range(NG):
            q_g.append(singles.tile([32, N // NG], FP, tag=f"q{h}", name=f"q{h}"))
            nc.vector.memset(q_g[h], -1.0)                      # pad rows never win
            QT_g.append(singles.tile([32, N // NG], FP, tag=f"QT{h}", name=f"QT{h}"))
            colmax_g.append(singles.tile([32, NGC], FP, tag=f"cm{h}", name=f"cm{h}"))
            maskT_g.append(singles.tile([32, NGC * P], FP, tag=f"mT{h}", name=f"mT{h}"))
            WT_g.append(singles.tile([128, NGK * M3], FP, tag=f"WT{h}", name=f"WT{h}"))
        rr = singles.tile([128, NT], FP, tag="rr")              # 1/max(||f||,eps)
        U = singles.tile([32, D + 4], FP, tag="U")              # current/updated protos
        Pn = singles.tile([32, D], FP, tag="Pn")                # normalized protos
        PnT = singles.tile([128, DT * P], FP, tag="PnT")        # sim lhsT
        PaT = singles.tile([128, DT * MA], FP, tag="PaT")       # final sim_map lhsT
        Pall = singles.tile([MA, D], FP, tag="Pall")
        S_al = singles.tile([32, D], FP, tag="S_al")
        msum_al = singles.tile([32, 1], FP, tag="msum_al")
        scratch = singles.tile([32, D], FP, tag="scratch")
        tau = singles.tile([32, 1], FP, tag="tau")
        zm8 = singles.tile([32, NB], FP, tag="zm8")             # per-bank sim maxes
        Z8 = singles.tile([32, NB], FP, tag="Z8")               # per-bank exp sums
        hmax = singles.tile([32, NG], FP, tag="hmax")
        hbias = singles.tile([32, NG], FP, tag="hbias")
        corr = singles.tile([32, NG], FP, tag="corr")
        scl = singles.tile([32, NG], FP, tag="scl")
        Zg = singles.tile([32, NG], FP, tag="Zg")
        st = {k: singles.tile([32, 1], FP, tag=f"st_{k}", name=f"st_{k}")
              for k in ("smax", "s", "gbias", "Z", "invZ", "ss", "nrm", "rp",
                        "dd", "m1", "mrec", "gate", "d1")}

        def build_FT(src_ap, store_r):
            """Stream [N,D] rows from DRAM, l2-normalize, transpose into FT."""
            for t in range(NT):
                w = win.tile([128, D], FP, tag="wtile")
                nc.sync.dma_start(out=w, in_=src_ap[t * 128:(t + 1) * 128, :])
                bs = small.tile([128, D], FP, tag="bscr")
                bq = small.tile([128, 1], FP, tag="bssq")
                br = small.tile([128, 1], FP, tag="br")
                nc.scalar.activation(out=bs, in_=w, func=AF.Square, accum_out=bq)
                nc.scalar.sqrt(out=bq, in_=bq)
                nc.vector.tensor_scalar_max(out=bq, in0=bq, scalar1=EPS)
                nc.vector.reciprocal(out=br, in_=bq)
                nc.scalar.mul(out=w, in_=w, mul=br)
                if store_r:
                    nc.vector.tensor_copy(out=rr[:, t:t + 1], in_=br)
                for j in range(DT):
                    pt = ps.tile([128, 128], FP, tag="ps1")
                    nc.tensor.transpose(pt, w[:, j * 128:(j + 1) * 128], ident)
                    nc.vector.tensor_copy(out=FTv[:, j, t * 128:(t + 1) * 128], in_=pt)

        def norm_chain(n_rows, src, dstPn, dstPnT, id_sl):
            """src [n_rows, D] -> normalized dstPn and transposed dstPnT."""
            nc.scalar.activation(out=scratch[0:n_rows, :], in_=src, func=AF.Square,
                                 accum_out=st["ss"][0:n_rows])
            nc.scalar.sqrt(out=st["nrm"][0:n_rows], in_=st["ss"][0:n_rows])
            nc.vector.tensor_scalar_max(out=st["nrm"][0:n_rows],
                                        in0=st["nrm"][0:n_rows], scalar1=EPS)
            nc.vector.reciprocal(out=st["rp"][0:n_rows], in_=st["nrm"][0:n_rows])
            nc.scalar.mul(out=dstPn[0:n_rows, :], in_=src, mul=st["rp"][0:n_rows])
            for j in range(DT):
                pt = ps.tile([128, n_rows], FP, tag="ps1")
                nc.tensor.transpose(pt, dstPn[0:n_rows, j * 128:(j + 1) * 128], id_sl)
                nc.vector.tensor_copy(out=dstPnT[:, j * n_rows:(j + 1) * n_rows], in_=pt)

        id10 = ident[0:P, 0:P]
        idMA = ident[0:MA, 0:MA]

        for b in range(nobj):
            build_FT(feats_in[b], store_r=True)
            nc.sync.dma_start(out=U[0:P, 0:D], in_=protos_in[b])
            nc.vector.memset(tau[0:P], TAU0)

            for t in range(n_iters):
                norm_chain(P, U[0:P, 0:D], Pn, PnT, id10)

                if t > 0:
                    # density/tau from previous iteration's S, msum and new Pn
                    nc.vector.tensor_mul(out=scratch[0:P, :], in0=Pn[0:P, :],
                                         in1=S_al[0:P, :])
                    nc.vector.tensor_reduce(out=st["dd"][0:P], in_=scratch[0:P, :],
                                            axis=AXX, op=ALU.add)
                    nc.vector.tensor_scalar_max(out=st["m1"][0:P], in0=msum_al[0:P],
                                                scalar1=1.0)
                    nc.vector.reciprocal(out=st["mrec"][0:P], in_=st["m1"][0:P])
                    nc.vector.tensor_scalar_min(out=st["gate"][0:P], in0=msum_al[0:P],
                                                scalar1=1.0)
                    nc.vector.tensor_mul(out=st["d1"][0:P], in0=st["dd"][0:P],
                                         in1=st["mrec"][0:P])
                    nc.vector.tensor_mul(out=st["d1"][0:P], in0=st["d1"][0:P],
                                         in1=st["gate"][0:P])
                    nc.vector.tensor_scalar(out=tau[0:P], in0=st["d1"][0:P],
                                            scalar1=-1.0, scalar2=1.0,
                                            op0=ALU.mult, op1=ALU.add)
                    nc.vector.tensor_scalar_max(out=tau[0:P], in0=tau[0:P],
                                                scalar1=TAU_MIN)

                # s = 1/(TEMP*tau), capped: beyond ~1e4 the softmax is already
                # an exact one-hot on this data, while uncapped s amplifies the
                # HW FMA cancellation residual of (sim*s - smax*s) to +-1e4,
                # overflowing exp (CoreSim's separate-rounding matmul hides it).
                nc.vector.tensor_scalar_mul(out=st["s"][0:P], in0=tau[0:P],
                                            scalar1=TEMP)
                nc.vector.reciprocal(out=st["s"][0:P], in_=st["s"][0:P])
                nc.vector.tensor_scalar_min(out=st["s"][0:P], in0=st["s"][0:P],
                                            scalar1=1.0e4)

                # ---- sim matmul + grouped online softmax ----
                for g in range(NG):
                    banks = range(g * GRP, min((g + 1) * GRP, NB))
                    pss = []
                    for nb in banks:
                        p_ = ps.tile([P, 512], FP, tag="ps1")
                        for kd in range(DT):
                            nc.tensor.matmul(p_, PnT[:, kd * P:(kd + 1) * P],
                                             FTv[:, kd, nb * 512:(nb + 1) * 512],
                                             start=(kd == 0), stop=(kd == DT - 1))
                        nc.vector.tensor_reduce(out=zm8[0:P, nb:nb + 1], in_=p_,
                                                axis=AXX, op=ALU.max)
                        pss.append(p_)
                    nc.vector.tensor_reduce(
                        out=hmax[0:P, g:g + 1],
                        in_=zm8[0:P, g * GRP:min((g + 1) * GRP, NB)],
                        axis=AXX, op=ALU.max)
                    nc.vector.tensor_scalar(out=hbias[0:P, g:g + 1],
                                            in0=hmax[0:P, g:g + 1],
                                            scalar1=st["s"][0:P], scalar2=-1.0,
                                            op0=ALU.mult, op1=ALU.mult)
                    for p_, nb in zip(pss, banks):
                        qsl = q_g[g][0:P, (nb - g * GRP) * 512:
                                     (nb - g * GRP + 1) * 512]
                        nc.scalar.activation(out=qsl, in_=p_, func=AF.Exp,
                                             bias=hbias[0:P, g:g + 1],
                                             scale=st["s"][0:P],
                                             accum_out=Z8[0:P, nb:nb + 1])
                    nc.vector.tensor_reduce(
                        out=Zg[0:P, g:g + 1],
                        in_=Z8[0:P, g * GRP:min((g + 1) * GRP, NB)],
                        axis=AXX, op=ALU.add)
                # global max + per-group correction, q *= corr_g/Z
                nc.vector.tensor_reduce(out=st["smax"][0:P], in_=hmax[0:P, 0:NG],
                                        axis=AXX, op=ALU.max)
                nc.vector.tensor_scalar(out=st["gbias"][0:P], in0=st["smax"][0:P],
                                        scalar1=st["s"][0:P], scalar2=-1.0,
                                        op0=ALU.mult, op1=ALU.mult)
                nc.scalar.activation(out=corr[0:P, 0:NG], in_=hmax[0:P, 0:NG],
                                     func=AF.Exp, bias=st["gbias"][0:P],
                                     scale=st["s"][0:P])
                nc.vector.tensor_mul(out=Zg[0:P, 0:NG], in0=Zg[0:P, 0:NG],
                                     in1=corr[0:P, 0:NG])
                nc.vector.tensor_reduce(out=st["Z"][0:P], in_=Zg[0:P, 0:NG],
                                        axis=AXX, op=ALU.add)
                nc.vector.reciprocal(out=st["invZ"][0:P], in_=st["Z"][0:P])
                nc.vector.tensor_scalar_mul(out=scl[0:P, 0:NG], in0=corr[0:P, 0:NG],
                                            scalar1=st["invZ"][0:P])
                for g in range(NG):
                    nc.scalar.mul(out=q_g[g][0:P, :], in_=q_g[g][0:P, :],
                                  mul=scl[0:P, g:g + 1])

                # ---- transpose weights + mask, per softmax group, pipelined ----
                last = (t == n_iters - 1)
                MU = P if last else M3
                pus = [ps.tile([M3, d1 - d0], FP, tag="pupd", bufs=3, name=f"pu{i}")
                       for i, (d0, d1) in enumerate(DSPLITS)]
                puc = (ps.tile([M3, 1], FP, tag="pupd", bufs=3, name="puc")
                       if not last else None)
                for h in range(NG):
                    QT = QT_g[h]
                    QTv = QT.rearrange("p (c j) -> p c j", j=32)
                    QTs = QT.rearrange("p (k r) -> p k r", r=128)
                    colmax = colmax_g[h]
                    maskT = maskT_g[h]
                    maskTv = maskT.rearrange("p (c j) -> p c j", j=P)
                    maskTs = maskT.rearrange("p (k d j) -> p k d j", d=4, j=P)
                    WTv = WT_g[h].rearrange("p (k m) -> p k m", m=M3)
                    nc.vector.transpose(out=QT, in_=q_g[h])
                    nc.vector.tensor_reduce(out=colmax, in_=QTv[:, :, 0:P],
                                            axis=AXX, op=ALU.max)
                    cm_b = bass.AP(tensor=colmax.tensor, offset=colmax.offset,
                                   ap=[colmax.ap[0], colmax.ap[1], [0, P]])
                    nc.vector.tensor_tensor(out=maskTv, in0=QTv[:, :, 0:P],
                                            in1=cm_b, op=ALU.is_equal)
                    nc.vector.tensor_mul(out=QTv[:, :, 0:P], in0=QTv[:, :, 0:P],
                                         in1=maskTv)
                    # PE warmers pinned to the chain: keep the HAM clock alive
                    pw = ps.tile([128, 32], FP, tag="ps1", name="pw")
                    nc.tensor.transpose(pw, QT[0:32, 0:128], ident[0:32, 0:32])
                    for d in range(4):
                        nc.sync.dma_start(out=WTv[d * 32:(d + 1) * 32, :, 0:P],
                                          in_=QTs[:, :, d * 32:d * 32 + P])
                        if not last:
                            nc.sync.dma_start(
                                out=WTv[d * 32:(d + 1) * 32, :, 2 * P:3 * P],
                                in_=maskTs[:, :, d, :])
                    if not last:
                        rr_sl = rr[:, h * NGK:(h + 1) * NGK]
                        rr_b = bass.AP(tensor=rr_sl.tensor, offset=rr_sl.offset,
                                       ap=[rr_sl.ap[0], rr_sl.ap[1], [0, P]])
                        nc.vector.tensor_tensor(out=WTv[:, :, P:2 * P],
                                                in0=WTv[:, :, 2 * P:3 * P],
                                                in1=rr_b, op=ALU.mult)
                    # ---- update matmul chunks of this group (raw feats HBM) ----
                    for kk in range(NGK):
                        k = h * NGK + kk
                        w = win.tile([128, D], FP, tag="wtile")
                        nc.sync.dma_start(out=w,
                                          in_=feats_in[b, k * 128:(k + 1) * 128, :])
                        lh = WTv[:, kk, 0:MU]
                        for pu, (d0, d1) in zip(pus, DSPLITS):
                            nc.tensor.matmul(pu[0:MU], lh, w[:, d0:d1],
                                             start=(k == 0), stop=(k == NT - 1))
                        if puc is not None:
                            nc.tensor.matmul(puc[0:MU], lh, ones,
                                             start=(k == 0), stop=(k == NT - 1))
                for pu, (d0, d1) in zip(pus, DSPLITS):
                    nc.vector.tensor_copy(out=U[0:MU, d0:d1], in_=pu[0:MU])
                if puc is not None:
                    nc.vector.tensor_copy(out=U[0:MU, D:D + 1], in_=puc[0:MU])
                    # partition-shift S and msum down to rows 0..P
                    nc.sync.dma_start(out=S_al[0:P, 0:D], in_=U[P:2 * P, 0:D])
                    nc.sync.dma_start(out=msum_al[0:P], in_=U[2 * P:3 * P, D:D + 1])

                if dbg_out is not None:
                    for di, dsrc in enumerate((tau, st["s"], st["smax"], st["Z"],
                                               msum_al, st["dd"], st["d1"],
                                               st["rp"])):
                        nc.sync.dma_start(out=dbg_out[b, t, di, :], in_=dsrc[:, 0])

            # object epilogue: final protos out + gather normalized rows
            nc.sync.dma_start(out=protos_out[b], in_=U[0:P, 0:D])
            norm_chain(P, U[0:P, 0:D], Pn, PnT, id10)
            nc.sync.dma_start(out=Pall[b * P:(b + 1) * P, :], in_=Pn[0:P, :])

        # ---- final sim_map against feats_org ----
        build_FT(forg_in[:, :], store_r=False)
        for j in range(DT):
            pt = ps.tile([128, MA], FP, tag="ps1")
            nc.tensor.transpose(pt, Pall[:, j * 128:(j + 1) * 128], idMA)
            nc.vector.tensor_copy(out=PaT[:, j * MA:(j + 1) * MA], in_=pt)
        smv = simmap_out.rearrange("b p n -> (b p) n")
        for nb in range(NB):
            p_ = ps.tile([MA, 512], FP, tag="ps1")
            for kd in range(DT):
                nc.tensor.matmul(p_, PaT[:, kd * MA:(kd + 1) * MA],
                                 FTv[:, kd, nb * 512:(nb + 1) * 512],
                                 start=(kd == 0), stop=(kd == DT - 1))
            sg = small.tile([MA, 512], FP, tag="smstage")
            nc.vector.tensor_copy(out=sg, in_=p_)
            nc.sync.dma_start(out=smv[:, nb * 512:(nb + 1) * 512], in_=sg)

    return nc


def legalize_waits(raw: bytes) -> bytes:
    """Split multi-wait instructions into single-wait NoOp prefixes.

    The walrus build in this environment rejects any instruction whose
    sync_info carries more than one on_wait entry ("Too many sync wait
    commands"), but Tile freely emits several.  Hoisting all but the last
    wait onto preceding same-engine NoOps is semantically identical under
    in-order per-engine execution.
    """
    import json

    data = json.loads(raw)
    n = 0
    for fn in data.get("functions", []):
        for blk in fn.get("blocks", []):
            out = []
            for inst in blk.get("instructions", []):
                si = inst.get("sync_info")
                waits = (si or {}).get("on_wait") or []
                if len(waits) > 1 and inst.get("engine") not in (None, "Unassigned"):
                    for w in waits[:-1]:
                        out.append({
                            "debug": inst.get("debug", 0),
                            "engine": inst["engine"],
                            "ins": [], "outs": [],
                            "name": f"WS-{n}",
                            "opcode": "NoOp",
                            "text_hint": "waitsplit",
                            "sync_info": {"on_wait": [w], "on_update": []},
                        })
                        n += 1
                    si["on_wait"] = [waits[-1]]
                out.append(inst)
            blk["instructions"] = out
    return json.dumps(data).encode()


_CACHE = {}


def _get_nc(key):
    if key not in _CACHE:
        nc = build_kernel(*key)
        legalized = legalize_waits(nc.to_json_bytes())
        nc.to_json_bytes = lambda: legalized
        _CACHE[key] = nc
    return _CACHE[key]


def kernel(prototypes, feats, feats_org):
    from concourse.bass_utils import run_bass_kernel_spmd

    B, P, D = prototypes.shape
    N = feats.shape[1]
    n_cores = 8
    nobj = B // n_cores
    nc = _get_nc((nobj, N, D, P, 5))
    eye = np.eye(128, dtype=np.float32)
    in_maps = []
    for c in range(n_cores):
        in_maps.append({
            "prototypes": np.ascontiguousarray(
                prototypes[c * nobj:(c + 1) * nobj], dtype=np.float32),
            "feats": np.ascontiguousarray(
                feats[c * nobj:(c + 1) * nobj], dtype=np.float32),
            "feats_org": np.ascontiguousarray(feats_org, dtype=np.float32),
            "identity": eye,
        })
    res = run_bass_kernel_spmd(nc, in_maps, list(range(n_cores))).results
    protos = np.concatenate([res[c]["protos_out"] for c in range(n_cores)], 0)
    sim_map = np.concatenate([res[c]["sim_map_out"] for c in range(n_cores)], 0)
    return protos.reshape(B * P, D), sim_map.reshape(B * P, N)
